# revision 126
# baseline (speedup 1.0000x reference)
"""NormLinearAttention kernel for 8 Trainium2 NeuronCores (fp8 build).

Strategy:
  Kernel A: head-parallel (core c owns head c, both batches): q/k/v AND
    the u gate projection for the head's channel slice, plus the full
    linear-attention pipeline. Projections use compensated fp8 DoubleRow
    matmuls: operands split host-side into fp8 hi + lo residual
    (x = x8 + dx8, 64w = w8 + dw8); each 1024-contraction is 12 DoubleRow
    matmuls (x8w8 + dw8x8 + w8dx8, 4 e-pairs each, K=256/instr at 0.5
    cyc/row) -> 25% fewer PE cycles than bf16 at ~bf16 accuracy. The /64
    weight prescale folds into the silu/copy activation scales.
    Linear attention with mask = exp(s*tril):
        out_i = (e^s-1)*causal_i + full_i
    chunked at 256 (pairs of 128 q-blocks); the running state P
    accumulates in one persistent PSUM bank (start=False chains) shared
    by both batches, and Q_m = (e^s-1)*P_m is snapshotted via Act copies
    with the per-core es1 scale.
  Host reshard (elementwise glue only): computes the SRMSNorm and gate
    fold z = (u + bu) * y * rsqrt(mean(y^2)+eps), splits z into fp8
    hi/lo, and re-tiles token-parallel.
  Kernel B: token-parallel (512 tokens/core) out-projection only, 8
    interleaved open PSUM groups whose terms are emitted in operand
    stream-arrival order; stage copies alternate Act/DVE.

Inputs are packed into few large DMAs (each costs ~625ns of shared
HWDGE); x streams as x8(t+1)-ahead-of-dx8(t) so two thirds of every
projection group can run before its residual half lands.
"""

import sys

sys.path.insert(0, "/opt/trn_rl_repo")

import numpy as np
import ml_dtypes

import bass_rust
import concourse.bass as bass
import concourse.mybir as mybir
import concourse.tile as tile
from concourse.bass_utils import run_bass_kernel_spmd

F32 = mybir.dt.float32
BF16 = mybir.dt.bfloat16
FP8 = mybir.dt.float8e4
AF = mybir.ActivationFunctionType
ALU = mybir.AluOpType
DR = mybir.MatmulPerfMode.DoubleRow

B, N, D = 2, 2048, 1024
H = 8
HD = 128
NC = 8
TT = 512           # projection token tile
NB = N // 128      # 16 token blocks per batch
NS = N // 256      # 8 state chunks per batch
EPS = 1e-6
WS = 64.0          # fp8 weight prescale
BF_NP = ml_dtypes.bfloat16
F8_NP = ml_dtypes.float8_e4m3


def _split_multi_waits(nc, max_waits=1):
    """walrus accepts one sync wait per instruction; Tile emits several.
    Insert same-engine NoOps each carrying one of the extra waits."""
    ctr = 0
    for func in nc.m.functions:
        for blk in func.blocks:
            out = []
            changed = False
            for inst in blk.instructions:
                si = inst.sync_info
                if si is not None and len(si.on_wait) > max_waits:
                    waits = list(si.on_wait)
                    for w in waits[:-max_waits]:
                        nop = bass_rust.InstNoOp(name=f"WSPLIT-{ctr}", ins=[], outs=[])
                        ctr += 1
                        nop.engine = inst.engine
                        nop.sync_info = mybir.SyncInfo(on_wait=[w], on_update=[])
                        out.append(nop)
                    inst.sync_info = mybir.SyncInfo(
                        on_wait=waits[-max_waits:], on_update=list(si.on_update)
                    )
                    changed = True
                out.append(inst)
            if changed:
                blk.instructions = out
    return ctr


def build_kernel_a(dbg=False):
    nc = bass.Bass("TRN2", target_bir_lowering=False, debug=False, num_devices=NC)
    # packed inputs: every extra DMA costs ~625ns on the shared HWDGE, so
    # tensors are fused host-side into few large transfers
    xpk_in = nc.dram_tensor("xpk", [B, 4, 128, 2 * 8 * TT], FP8,
                            kind="ExternalInput").ap()   # x8 || dx8 per tile
    wpk_in = nc.dram_tensor("wpk", [128, 8192], FP8,
                            kind="ExternalInput").ap()   # wq8|wkv8|wu8|d...
    tabT_in = nc.dram_tensor("tabT", [128, 2 * N], BF16,
                             kind="ExternalInput").ap()  # cosT || sinT
    tabTT_in = nc.dram_tensor("tabTT", [128, 2 * N], BF16,
                              kind="ExternalInput").ap()  # cosTT || sinTT
    bqm_in = nc.dram_tensor("bqm", [128, 2 + 384], F32,
                            kind="ExternalInput").ap()   # bqes || mask3
    bkvid_in = nc.dram_tensor("bkvid", [128, 640], BF16, kind="ExternalInput").ap()
    y_out = nc.dram_tensor("y", [B, 128, N], BF16, kind="ExternalOutput").ap()
    u_out = nc.dram_tensor("u", [B, 128, N], BF16, kind="ExternalOutput").ap()
    dbg_outs = {}
    if dbg:
        for nm in ("qec", "qes", "kec", "kes", "knc", "kns", "ksn", "ksT"):
            dbg_outs[nm] = nc.dram_tensor(
                "d_" + nm, [128, N], BF16, kind="ExternalOutput").ap()
        dbg_outs["kvnat"] = nc.dram_tensor(
            "d_kvnat", [128, NB * 256], BF16, kind="ExternalOutput").ap()
        dbg_outs["QB"] = nc.dram_tensor(
            "d_QB", [128, NS * 256], BF16, kind="ExternalOutput").ap()

    with tile.TileContext(nc) as tc:
        with tc.tile_pool(name="consts", bufs=1) as cp, \
             tc.tile_pool(name="big", bufs=1) as bp, \
             tc.tile_pool(name="xt", bufs=7) as xp, \
             tc.tile_pool(name="qsil", bufs=3) as qsp, \
             tc.tile_pool(name="em", bufs=4) as emp:
            # wpk first half (wq+wkv hi), then the first x tile split so the
            # first projection matmuls start as early as possible
            xt00 = xp.tile([128, 2 * 8 * TT], FP8, tag="xt")
            wpk = cp.tile([128, 8192], FP8)
            tabT = cp.tile([128, 2 * N], BF16)
            tabTT = cp.tile([128, 2 * N], BF16)
            bqm = cp.tile([128, 2 + 384], F32)
            bkvid = cp.tile([128, 640], BF16)
            bz = cp.tile([128, 1], F32)
            wqt = wpk[:, 0:1024]
            wkvt = wpk[:, 1024:3072]
            wut = wpk[:, 3072:4096]
            dwqt = wpk[:, 4096:5120]
            dwkvt = wpk[:, 5120:7168]
            dwut = wpk[:, 7168:8192]
            cosT = tabT[:, 0:N]
            sinT = tabT[:, N:2 * N]
            cosTT = tabTT[:, 0:N]
            sinTT = tabTT[:, N:2 * N]
            bq = bqm[:, 0:1]
            es1 = bqm[:, 1:2]
            mask3 = bqm[:, 2:386]
            bkv = bkvid[:, 0:512]
            ident = bkvid[:, 512:640]
            # tiny bias/scale tensors first: the first silu and the first kv
            # consumer block on them, so they must not queue behind the bulk
            nc.sync.dma_start(bqm[:], bqm_in)
            nc.sync.dma_start(wpk[:, 0:4096], wpk_in[:, 0:4096])
            nc.sync.dma_start(bkvid[:], bkvid_in)
            nc.sync.dma_start(xt00[:, 0:4 * TT], xpk_in[0][0][:, 0:4 * TT])
            nc.sync.dma_start(xt00[:, 4 * TT:8 * TT], xpk_in[0][0][:, 4 * TT:8 * TT])
            nc.sync.dma_start(wpk[:, 4096:8192], wpk_in[:, 4096:8192])
            nc.vector.memset(bz[:], 0.0)
            # warm the PE p-state ramp while the first DMAs land: throwaway
            # matmuls on a zeroed tile so real matmuls start at full speed
            wz = qsp.tile([128, TT], BF16, tag="warm")
            nc.vector.memset(wz[:], 0.0)

            st = []  # per-batch persistent tensors
            for b in range(B):
                d = {}
                for nm, shape, dt in (
                    ("kvnat", [128, NB * 256], BF16),
                    ("ksn", [128, N], BF16),
                    ("ksT", [128, N], BF16),
                    ("qec", [128, N], BF16),
                    ("qes", [128, N], BF16),
                    ("kec", [128, N], BF16),
                    ("kes", [128, N], BF16),
                    ("knc", [128, N], BF16),
                    ("kns", [128, N], BF16),
                    ("QB", [128, NS * 256], BF16),
                    ("yst", [128, N], BF16),
                    ("ust", [128, N], BF16),
                ):
                    d[nm] = bp.tile(shape, dt, tag=f"{nm}{b}", name=f"{nm}{b}")
                st.append(d)

            def vn_blk(b, j):
                # v for token block j lives in the kv-paired tile
                return st[b]["kvnat"][:, j * 256 + 128:(j + 1) * 256]

            def phase1_tile0(pq, pkv, ptp, pup, xt):
                """Tile (b=0, t=0) with stage-interleaved emission: all six
                PSUM groups (q, 4 kv halves, u) advance term-by-term so the
                in-order PE queue runs every matmul whose operands have
                landed, instead of parking a group on a late DMA."""
                b, t = 0, 0
                s = st[0]
                tsl = slice(0, TT)
                xk = xt.rearrange("p (k e t) -> p k e t", k=2, e=8)
                wqr = wqt.rearrange("p (e o) -> p e o", e=8)
                dwqr = dwqt.rearrange("p (e o) -> p e o", e=8)
                wur = wut.rearrange("p (e o) -> p e o", e=8)
                dwur = dwut.rearrange("p (e o) -> p e o", e=8)
                wkvr = wkvt.rearrange("p (e kv) -> p e kv", e=8)
                dwkvr = dwkvt.rearrange("p (e kv) -> p e kv", e=8)
                psq = pq.tile([128, TT], F32, tag="psq")
                pskvs = [pkv.tile([128, 512], F32, tag="pskv", name=f"pskv0{j}")
                         for j in range(2)]
                psu = pup.tile([128, TT], F32, tag="pu")

                def em_hd(ps, wr_, kk, first, last):
                    for ep in range(4):
                        pe = slice(2 * ep, 2 * ep + 2)
                        nc.tensor.matmul(
                            ps[:], wr_[:, pe, :], xk[:, kk, pe, :],
                            start=(first and ep == 0), stop=(last and ep == 3),
                            perf_mode=DR)

                def em_kv(jb, ws_, kk, first, last):
                    pskv = pskvs[jb // 2]
                    osl = slice((jb % 2) * 256, (jb % 2 + 1) * 256)
                    tok = slice(jb * 128, (jb + 1) * 128)
                    for ep in range(4):
                        pe = slice(2 * ep, 2 * ep + 2)
                        nc.tensor.matmul(
                            pskv[:, osl], xk[:, kk, pe, tok], ws_[:, pe, :],
                            # only the bank's first half opens with start=True:
                            # the bank-level zero covers the second half
                            start=(first and ep == 0 and jb % 2 == 0),
                            stop=(last and ep == 3), perf_mode=DR)

                for si, (dwf, kk) in enumerate(((0, 0), (1, 0), (0, 1))):
                    first, last = si == 0, si == 2
                    em_hd(psq, dwqr if dwf else wqr, kk, first, last)
                    for jb in range(4):
                        em_kv(jb, dwkvr if dwf else wkvr, kk, first, last)
                    em_hd(psu, dwur if dwf else wur, kk, first, last)

                qsil = qsp.tile([128, TT], BF16, tag="qsil")
                nc.scalar.activation(qsil[:], psq[:], AF.Silu, bias=bq,
                                     scale=1.0 / WS)
                ptr4 = ptp.tile([128, 512], BF16, tag="ptr4")
                for jp in range(2):
                    jj0 = jp * 2
                    nc.vector.scalar_tensor_tensor(
                        s["kvnat"][:, jj0 * 256:(jj0 + 2) * 256], pskvs[jp][:],
                        1.0 / WS, bkv, ALU.mult, ALU.add)
                    kpair = s["kvnat"].rearrange("p (j kv c) -> p j kv c",
                                                 kv=2, c=128)
                    nc.scalar.activation(
                        s["ksn"][:, jj0 * 128:(jj0 + 2) * 128],
                        kpair[:, jj0:jj0 + 2, 0],
                        AF.Silu, bias=bz[:])
                    hsl = slice(jp * 256, (jp + 1) * 256)
                    nc.vector.tensor_mul(s["knc"][:, hsl], s["ksn"][:, hsl],
                                         cosTT[:, hsl])
                    nc.vector.tensor_mul(s["kns"][:, hsl], s["ksn"][:, hsl],
                                         sinTT[:, hsl])
                    for half in range(2):
                        jb = jp * 2 + half
                        nc.tensor.transpose(ptr4[:, jb * 128:(jb + 1) * 128],
                                            s["ksn"][:, jb * 128:(jb + 1) * 128],
                                            ident)
                nc.scalar.copy(s["ksT"][:, tsl], ptr4[:])
                nc.scalar.activation(s["ust"][:, tsl], psu[:], AF.Copy,
                                     scale=1.0 / WS)
                nc.vector.tensor_mul(s["kec"][:, tsl], s["ksT"][:, tsl], cosT[:, tsl])
                nc.vector.tensor_mul(s["kes"][:, tsl], s["ksT"][:, tsl], sinT[:, tsl])
                nc.vector.tensor_mul(s["qec"][:, tsl], qsil[:], cosT[:, tsl])
                nc.vector.tensor_mul(s["qes"][:, tsl], qsil[:], sinT[:, tsl])

            def phase1_tile(b, t, pq, pkv, ptp, pup, xt):
                s = st[b]
                tsl = slice(t * TT, (t + 1) * TT)
                xk = xt.rearrange("p (k e t) -> p k e t", k=2, e=8)
                wqr = wqt.rearrange("p (e o) -> p e o", e=8)
                dwqr = dwqt.rearrange("p (e o) -> p e o", e=8)
                wur = wut.rearrange("p (e o) -> p e o", e=8)
                dwur = dwut.rearrange("p (e o) -> p e o", e=8)
                wkvr = wkvt.rearrange("p (e kv) -> p e kv", e=8)
                dwkvr = dwkvt.rearrange("p (e kv) -> p e kv", e=8)
                # q projection: [hd, tok], compensated fp8 DoubleRow.
                # Term order (w8*x8, dw8*x8, w8*dx8): the first two-thirds of
                # every group need only the x8 half of the tile, which the
                # DMA stream sends one tile ahead of the dx8 halves.
                psq = pq.tile([128, TT], F32, tag="psq")
                ti = 0
                for wr_, kk in ((wqr, 0), (dwqr, 0), (wqr, 1)):
                    for ep in range(4):
                        pe = slice(2 * ep, 2 * ep + 2)
                        nc.tensor.matmul(
                            psq[:], wr_[:, pe, :], xk[:, kk, pe, :],
                            start=(ti == 0), stop=(ti == 11), perf_mode=DR)
                        ti += 1
                qsil = qsp.tile([128, TT], BF16, tag="qsil" if b == 0 else f"qsil1_{t}",
                                bufs=1 if b else None)
                nc.scalar.activation(qsil[:], psq[:], AF.Silu, bias=bq, scale=1.0 / WS)
                # k|v paired projection: [tok, k|v]; two token blocks share a
                # psum bank as sequential accumulation groups
                ptr4 = ptp.tile([128, 512], BF16, tag="ptr4")
                for jp in range(2):
                    pskv = pkv.tile([128, 512], F32, tag="pskv")
                    for half in range(2):
                        jb = jp * 2 + half
                        osl = slice(half * 256, (half + 1) * 256)
                        tok = slice(jb * 128, (jb + 1) * 128)
                        ti = 0
                        for kk, ws_ in ((0, wkvr), (0, dwkvr), (1, wkvr)):
                            for ep in range(4):
                                pe = slice(2 * ep, 2 * ep + 2)
                                nc.tensor.matmul(
                                    pskv[:, osl], xk[:, kk, pe, tok], ws_[:, pe, :],
                                    start=(ti == 0), stop=(ti == 11), perf_mode=DR)
                                ti += 1
                    jj0 = t * 4 + jp * 2
                    nc.vector.scalar_tensor_tensor(
                        s["kvnat"][:, jj0 * 256:(jj0 + 2) * 256], pskv[:],
                        1.0 / WS, bkv, ALU.mult, ALU.add)
                    kpair = s["kvnat"].rearrange("p (j kv c) -> p j kv c",
                                                 kv=2, c=128)
                    nc.scalar.activation(
                        s["ksn"][:, jj0 * 128:(jj0 + 2) * 128],
                        kpair[:, jj0:jj0 + 2, 0],
                        AF.Silu, bias=bz[:])
                    # state-path lrpe for this half right after its silu:
                    # the interleaved 2a chunk matmuls wait on these
                    hsl = slice(t * TT + jp * 256, t * TT + (jp + 1) * 256)
                    nc.vector.tensor_mul(s["knc"][:, hsl], s["ksn"][:, hsl],
                                         cosTT[:, hsl])
                    nc.vector.tensor_mul(s["kns"][:, hsl], s["ksn"][:, hsl],
                                         sinTT[:, hsl])
                    for half in range(2):
                        jb = jp * 2 + half
                        jj = t * 4 + jb
                        nc.tensor.transpose(ptr4[:, jb * 128:(jb + 1) * 128],
                                            s["ksn"][:, jj * 128:(jj + 1) * 128],
                                            ident)
                nc.scalar.copy(s["ksT"][:, tsl], ptr4[:])
                # u projection for this head's channel slice: feeds the host
                # z-fold between launches. Emitted after kv so the 2a chain's
                # inputs (knc/kns) are produced while u's matmuls run.
                psu = pup.tile([128, TT], F32, tag="pu")
                ti = 0
                for wr_, kk in ((wur, 0), (dwur, 0), (wur, 1)):
                    for ep in range(4):
                        pe = slice(2 * ep, 2 * ep + 2)
                        nc.tensor.matmul(
                            psu[:], wr_[:, pe, :], xk[:, kk, pe, :],
                            start=(ti == 0), stop=(ti == 11), perf_mode=DR)
                        ti += 1
                nc.scalar.activation(s["ust"][:, tsl], psu[:], AF.Copy,
                                     scale=1.0 / WS)
                if t % 2 == 1:
                    gsl = slice((t - 1) * TT, (t + 1) * TT)
                    nc.sync.dma_start(u_out[b][:, gsl], s["ust"][:, gsl])
                # q/kec/kes are only read in 2b.  For b=1 those muls are
                # deferred so they don't sit in the DVE queue ahead of
                # 2b(0)'s masked multiplies.

                def lrpe_2b():
                    nc.vector.tensor_mul(s["kec"][:, tsl], s["ksT"][:, tsl], cosT[:, tsl])
                    nc.vector.tensor_mul(s["kes"][:, tsl], s["ksT"][:, tsl], sinT[:, tsl])
                    nc.vector.tensor_mul(s["qec"][:, tsl], qsil[:], cosT[:, tsl])
                    nc.vector.tensor_mul(s["qes"][:, tsl], qsil[:], sinT[:, tsl])
                if b == 0:
                    lrpe_2b()
                    return None
                return lrpe_2b

            def phase2a_chunk(b, ms, stp):
                s = st[b]
                if ms > 0:
                    # snapshot Q_ms = (e^s-1)*P_ms before adding chunk ms
                    nc.scalar.activation(s["QB"][:, ms * 256:(ms + 1) * 256],
                                         stp[:], AF.Copy, scale=es1)
                # running state accumulates in the persistent PSUM bank.
                # start=True only on the very first matmul: it zeroes the
                # whole 2KB bank (both g halves), so g=1's first write and
                # every later chunk must accumulate (start=False) or the
                # bank-level pending-zero wipes the other half's history.
                for g in range(2):
                    kn = s["knc"] if g == 0 else s["kns"]
                    for s2 in range(2):
                        j = 2 * ms + s2
                        bsl = slice(j * 128, (j + 1) * 128)
                        nc.tensor.matmul(stp[:, g * 128:(g + 1) * 128],
                                         kn[:, bsl], vn_blk(b, j),
                                         start=(ms == 0 and s2 == 0 and g == 0),
                                         stop=(s2 == 1),
                                         skip_group_check=True)

            def phase2a_post(b, stp):
                s = st[b]
                # Q_0 = S_full; Q_m += S_full. The adds are independent and
                # bf16/SBUF, so DVE's 2x mode (194ns) beats Pool (603ns);
                # split across both so neither chain gates 2b's outputs.
                nc.scalar.activation(s["QB"][:, 0:256], stp[:], AF.Copy)
                for ms in range(1, NS):
                    msl = slice(ms * 256, (ms + 1) * 256)
                    nc.gpsimd.tensor_add(s["QB"][:, msl], s["QB"][:, msl],
                                         s["QB"][:, 0:256])

            def phase2b(b, pet, pot, interleave=None):
                s = st[b]

                def emit_et(ms):
                    etb = pet.tile([128, 512], F32, tag="etb", name=f"etb{ms}")
                    for s2 in range(2):
                        qb = 2 * ms + s2
                        qsl = slice(qb * 128, (qb + 1) * 128)
                        et = etb[:, s2 * 128:(s2 + 1) * 128]
                        nc.tensor.matmul(et, s["kec"][:, qsl], s["qec"][:, qsl],
                                         start=True, stop=False)
                        nc.tensor.matmul(et, s["kes"][:, qsl], s["qes"][:, qsl],
                                         start=False, stop=True)
                    bsl = slice(2 * ms * 128, (2 * ms + 1) * 128)
                    q1sl = slice((2 * ms + 1) * 128, (2 * ms + 2) * 128)
                    et2 = etb[:, 256:384]
                    nc.tensor.matmul(et2, s["kec"][:, bsl], s["qec"][:, q1sl],
                                     start=True, stop=False)
                    nc.tensor.matmul(et2, s["kes"][:, bsl], s["qes"][:, q1sl],
                                     start=False, stop=True)
                    # one masked multiply for all three energy blocks
                    em3 = emp.tile([128, 384], BF16, tag="em", name=f"em_{ms}")
                    nc.vector.tensor_mul(em3[:], etb[:, 0:384], mask3[:])
                    return em3

                def emit_ot(ms, em3):
                    qmsl = slice(ms * 256, ms * 256 + 128)
                    smsl = slice(ms * 256 + 128, (ms + 1) * 256)
                    otb = pot.tile([128, 256], F32, tag="otb", name=f"otb{ms}")
                    for s2 in range(2):
                        qb = 2 * ms + s2
                        qsl = slice(qb * 128, (qb + 1) * 128)
                        ot = otb[:, s2 * 128:(s2 + 1) * 128]
                        nc.tensor.matmul(ot, s["QB"][:, qmsl], s["qec"][:, qsl],
                                         start=True, stop=False)
                        nc.tensor.matmul(ot, s["QB"][:, smsl], s["qes"][:, qsl],
                                         start=False, stop=False)
                        if s2 == 1:
                            nc.tensor.matmul(ot, vn_blk(b, 2 * ms),
                                             em3[:, 256:384],
                                             start=False, stop=False)
                        nc.tensor.matmul(ot, vn_blk(b, qb),
                                         em3[:, s2 * 128:(s2 + 1) * 128],
                                         start=False, stop=True)
                    # stage via SBUF (DGE cannot read PSUM); ship pairs,
                    # the final ones singly to shorten the drain tail
                    osl = slice(2 * ms * 128, (2 * ms + 2) * 128)
                    nc.scalar.copy(s["yst"][:, osl], otb[:])
                    if ms % 2 == 1:
                        gsl = slice((ms // 2) * 512, (ms // 2 + 1) * 512)
                        nc.sync.dma_start(y_out[b][:, gsl], s["yst"][:, gsl])

                # software pipeline: energies run ahead of outputs; other
                # batches' deferred DVE work interleaves under the PE matmuls
                DEPTH = 2
                ems = {}
                for ms in range(NS):
                    ems[ms] = emit_et(ms)
                    if interleave and ms % 2 == 1:
                        interleave[ms // 2]()
                    if ms >= DEPTH:
                        emit_ot(ms - DEPTH, ems.pop(ms - DEPTH))
                for ms in range(NS - DEPTH, NS):
                    emit_ot(ms, ems.pop(ms))

            with tc.tile_pool(name="pq", bufs=2, space="PSUM") as pq, \
                 tc.tile_pool(name="pkv", bufs=3, space="PSUM") as pkv, \
                 tc.tile_pool(name="ptr", bufs=1, space="PSUM") as ptp, \
                 tc.tile_pool(name="pu", bufs=1, space="PSUM") as pup, \
                 tc.tile_pool(name="pst", bufs=1, space="PSUM") as pstp:
                # one state bank shared by both batches: b1's first matmul
                # (start=True) re-zeroes it after b0's snapshots are taken
                stp0 = pstp.tile([128, 256], F32, tag="stp", name="stp")
                stp = [stp0, stp0]
                # prefetch x in x8/dx8-interleaved order: tile t's first 8
                # matmuls per group need only the x8 half, so stream
                # x8(t+1) ahead of dx8(t); tables slot into the gaps by
                # first-use time (tabTT at tile0's kv, tabT by 2b)
                allx = [xt00]
                for i in range(1, 8):
                    xtn = xp.tile([128, 2 * 8 * TT], FP8, tag="xt", name=f"xt{i}")
                    allx.append(xtn)
                xts0, xts1 = allx[0:4], allx[4:8]
                srcs = [xpk_in[i // 4][i % 4] for i in range(8)]
                # prefetch x in x8/dx8-interleaved order: tile t's first 8
                # matmuls per group need only the x8 half, so stream
                # x8(t+1) ahead of dx8(t); tables slot into the gaps by
                # first-use time (tabTT at tile0's kv, tabT by 2b)
                nc.sync.dma_start(allx[1][:, 0:8 * TT], srcs[1][:, 0:8 * TT])
                nc.sync.dma_start(xt00[:, 8 * TT:16 * TT],
                                  srcs[0][:, 8 * TT:16 * TT])
                nc.sync.dma_start(tabTT[:], tabTT_in)
                for i in range(2, 8):
                    nc.sync.dma_start(allx[i][:, 0:8 * TT], srcs[i][:, 0:8 * TT])
                    nc.sync.dma_start(allx[i - 1][:, 8 * TT:16 * TT],
                                      srcs[i - 1][:, 8 * TT:16 * TT])
                    if i == 2:
                        nc.sync.dma_start(tabT[:], tabT_in)
                nc.sync.dma_start(allx[7][:, 8 * TT:16 * TT],
                                  srcs[7][:, 8 * TT:16 * TT])
                wp = pq.tile([128, TT], F32, tag="psq", name="warmps")
                for i in range(8):
                    nc.tensor.matmul(wp[:], wz[:, 0:128], wz[:],
                                     start=True, stop=True)
                # state chunks interleave with projection tiles, lagging one
                # chunk behind their producers so the Act snapshot + DVE lrpe
                # chain never stalls the in-order PE queue
                # state chunks lag their producer tile by one chunk so the
                # Act snapshot + DVE lrpe chain never stalls the in-order PE
                for t in range(4):
                    phase1_tile(0, t, pq, pkv, ptp, pup, xts0[t])
                    if t > 0:
                        phase2a_chunk(0, 2 * t - 1, stp[0])
                    phase2a_chunk(0, 2 * t, stp[0])
                phase2a_chunk(0, 7, stp[0])
                phase2a_post(0, stp[0])
                deferred = []
                for t in range(4):
                    deferred.append(
                        phase1_tile(1, t, pq, pkv, ptp, pup, xts1[t]))
                    if t > 0:
                        phase2a_chunk(1, 2 * t - 1, stp[1])
                    phase2a_chunk(1, 2 * t, stp[1])
                phase2a_chunk(1, 7, stp[1])
                phase2a_post(1, stp[1])
            with tc.tile_pool(name="pet", bufs=4, space="PSUM") as pet, \
                 tc.tile_pool(name="pot", bufs=4, space="PSUM") as pot:
                # b1's 2b-only lrpe muls run on DVE while 2b(0) computes
                phase2b(0, pet, pot)
                for fn in deferred:
                    fn()
                phase2b(1, pet, pot)
            if dbg:
                for nm, dst in dbg_outs.items():
                    nc.sync.dma_start(dst, st[0][nm][:])

    return nc


def build_kernel_b():
    """o-projection only: z = (u+bu)*yhat is folded host-side between the
    launches (elementwise glue on the reshard path) and arrives pre-split
    into fp8 hi/lo. out_t = z @ (WS*wo).T is shipped as WS*(out - out_b)."""
    nc = bass.Bass("TRN2", target_bir_lowering=False, debug=False, num_devices=NC)
    NT = B * N // NC  # 512 tokens per core
    z8_in = nc.dram_tensor("z8", [128, 8 * NT], FP8, kind="ExternalInput").ap()
    dz8_in = nc.dram_tensor("dz8", [128, 8 * NT], FP8, kind="ExternalInput").ap()
    wo8_in = nc.dram_tensor("wo8", [128, 64 * 128], FP8, kind="ExternalInput").ap()
    dwo8_in = nc.dram_tensor("dwo8", [128, 64 * 128], FP8,
                             kind="ExternalInput").ap()
    out_t = nc.dram_tensor("outT", [128, 8 * NT], BF16, kind="ExternalOutput").ap()

    with tile.TileContext(nc) as tc:
        with tc.tile_pool(name="ins", bufs=1) as ip, \
             tc.tile_pool(name="work", bufs=1) as wk, \
             tc.tile_pool(name="po", bufs=8, space="PSUM") as pop:
            z8 = ip.tile([128, 8 * NT], FP8)
            dz8 = ip.tile([128, 8 * NT], FP8)
            wo8 = ip.tile([128, 64 * 128], FP8)
            dwo8 = ip.tile([128, 64 * 128], FP8)
            # stream operands in first-use order of the staged group emission
            nc.sync.dma_start(z8[:], z8_in)
            nc.sync.dma_start(wo8[:, 0:1024], wo8_in[:, 0:1024])
            nc.sync.dma_start(wo8[:, 1024:2048], wo8_in[:, 1024:2048])
            nc.sync.dma_start(wo8[:, 2048:4096], wo8_in[:, 2048:4096])
            nc.sync.dma_start(dz8[:], dz8_in)
            nc.sync.dma_start(wo8[:, 4096:8192], wo8_in[:, 4096:8192])
            nc.sync.dma_start(dwo8[:, 0:4096], dwo8_in[:, 0:4096])
            nc.sync.dma_start(dwo8[:, 4096:8192], dwo8_in[:, 4096:8192])

            # warm the PE p-state ramp during the DMA lead-in (rotates into
            # the po pool: its bank is recycled by the 8th o-group). Sized to
            # end right as z8+wo8h1 land: each costs ~213ns at mid p-state,
            # and 16 of them cover the ~3.4us lead-in exactly.
            wz = wk.tile([128, NT], BF16, tag="warm")
            nc.vector.memset(wz[:], 0.0)
            wp = pop.tile([128, NT], F32, tag="po", name="warmps")
            for i in range(15):
                nc.tensor.matmul(wp[:, 0:NT // 2], wz[:, 0:128],
                                 wz[:, 0:NT // 2], start=True, stop=True)

            ostage = wk.tile([128, 8 * NT], BF16, tag="ostage")
            z8r = z8.rearrange("p (e t) -> p e t", e=8)
            dz8r = dz8.rearrange("p (e t) -> p e t", e=8)
            wo8r = wo8.rearrange("p (u e o) -> p u e o", u=8, e=8)
            dwo8r = dwo8.rearrange("p (u e o) -> p u e o", u=8, e=8)

            # all 8 PSUM groups stay open; terms are emitted globally in the
            # order their operands stream in, so the in-order PE queue never
            # parks behind a late tensor
            pss = [pop.tile([128, NT], F32, tag="po", name=f"po{oc}")
                   for oc in range(8)]

            def o_term(ocs, ws_, zs_, first=False, last=False,
                       ship_at=(1, 3, 5, 7)):
                for oc in ocs:
                    ps = pss[oc]
                    for ep in range(4):
                        pe = slice(2 * ep, 2 * ep + 2)
                        nc.tensor.matmul(
                            ps[:], ws_[:, oc, pe, :], zs_[:, pe, :],
                            start=(first and ep == 0), stop=(last and ep == 3),
                            perf_mode=DR)
                    if last:
                        # stage copies alternate Act/DVE so the two chains
                        # drain in parallel instead of serializing on Act
                        osl = slice(oc * NT, (oc + 1) * NT)
                        if oc % 2 == 0:
                            nc.scalar.copy(ostage[:, osl], ps[:])
                        else:
                            nc.vector.tensor_scalar_mul(ostage[:, osl],
                                                        ps[:], 1.0)
                        # early blocks ship in pairs (each DMA holds the
                        # shared HWDGE ~625ns, so fewer slots ahead of the
                        # final ship); the last two ship singly
                        if oc in ship_at:
                            g0 = (oc // 2) * 2
                            gsl = slice(g0 * NT, (g0 + 2) * NT)
                            nc.sync.dma_start(out_t[:, gsl], ostage[:, gsl])

            q0, q1, q2 = range(0, 2), range(2, 4), range(4, 8)
            o_term(q0, wo8r, z8r, first=True)   # needs z8 + wo8 q1
            o_term(q1, wo8r, z8r, first=True)   # + wo8 q2
            o_term(q0, wo8r, dz8r)              # + dz8
            o_term(q1, wo8r, dz8r)
            o_term(q2, wo8r, z8r, first=True)   # + wo8 h2
            o_term(q0, dwo8r, z8r, last=True)   # + dwo8 h1
            o_term(q1, dwo8r, z8r, last=True)
            o_term(q2, wo8r, dz8r)
            o_term(q2, dwo8r, z8r, last=True)   # + dwo8 h2

    return nc


_CACHE = {}


def _bf(a):
    return np.ascontiguousarray(a.astype(BF_NP))


def _f8split(a):
    """fp8 hi + fp8 residual decomposition of a float32 array."""
    hi = a.astype(F8_NP)
    lo = (a - hi.astype(np.float32)).astype(F8_NP)
    return np.ascontiguousarray(hi), np.ascontiguousarray(lo)


def kernel(x, slope_rate, qkvu_w, qkvu_b, out_w, out_b, theta):
    x = np.asarray(x, np.float32)
    slope_rate = np.asarray(slope_rate, np.float32)
    qkvu_w = np.asarray(qkvu_w, np.float32)
    qkvu_b = np.asarray(qkvu_b, np.float32)
    out_w = np.asarray(out_w, np.float32)
    out_b = np.asarray(out_b, np.float32)
    theta = np.asarray(theta, np.float32)

    # x in [b, t, p, e*512+j] layout: xh[b,t,p,e*512+j] = x[b, t*512+j, e*128+p]
    xh = (x.reshape(B, 4, TT, 8, 128).transpose(0, 1, 4, 3, 2)
          .reshape(B, 4, 128, 8 * TT))
    x8h, dx8h = _f8split(xh)
    xpk = np.ascontiguousarray(np.concatenate([x8h, dx8h], axis=-1))
    idx = np.arange(N, dtype=np.float32)
    ident = np.eye(128, dtype=np.float32)

    in_maps_a = []
    for c in range(NC):
        th = theta[c, 0].astype(np.float32)[:, None] * idx[None, :]  # [128, N]
        es = np.exp(slope_rate[c, 0, 0]).astype(np.float32)
        es1 = np.float32(es - 1.0)
        sl = slice(c * HD, (c + 1) * HD)
        wq = qkvu_w[0 * D:1 * D][sl]   # [128, D]
        wk = qkvu_w[1 * D:2 * D][sl]
        wv = qkvu_w[2 * D:3 * D][sl]
        # wq image: [p, e*128+o] = wq[o, e*128+p]
        wq_img = wq.T.reshape(8, 128, 128).transpose(1, 0, 2).reshape(128, 8 * 128)
        # wkv image: [p, e*256 + (k|v 128+o)]
        wkv_img = np.concatenate(
            [wk.T.reshape(8, 128, 1, 128), wv.T.reshape(8, 128, 1, 128)], axis=2
        ).transpose(1, 0, 2, 3).reshape(128, 8 * 256)
        wu_h = qkvu_w[3 * D:4 * D][sl]
        wu_img = wu_h.T.reshape(8, 128, 128).transpose(1, 0, 2).reshape(
            128, 8 * 128)
        wq8, dwq8 = _f8split(wq_img * WS)
        wkv8, dwkv8 = _f8split(wkv_img * WS)
        wu8h, dwu8h = _f8split(wu_img * WS)
        cosv = np.cos(th)  # [hd, pos]
        sinv = np.sin(th)
        # token-layout tables: [p, jj*128 + d] = f(theta_d * (jj*128+p))
        cosvT = cosv.T.reshape(NB, 128, 128).transpose(1, 0, 2).reshape(128, N)
        sinvT = sinv.T.reshape(NB, 128, 128).transpose(1, 0, 2).reshape(128, N)
        m0 = (np.arange(128)[:, None] <= np.arange(128)[None, :]).astype(np.float32)
        m3 = np.concatenate(
            [m0 * es1, m0 * es1, np.full((128, 128), es1, np.float32)], axis=1)
        bkv_img = np.broadcast_to(
            np.concatenate([qkvu_b[1 * D:2 * D][sl], qkvu_b[2 * D:3 * D][sl]] * 2),
            (128, 512))
        bkvid_img = np.concatenate([bkv_img, ident], axis=1)
        bqes_img = np.stack(
            [qkvu_b[0 * D:1 * D][sl], np.full(128, es1, np.float32)], axis=1)
        in_maps_a.append({
            "xpk": xpk,
            "wpk": np.ascontiguousarray(
                np.concatenate([wq8, wkv8, wu8h, dwq8, dwkv8, dwu8h], axis=1)),
            "tabT": _bf(np.concatenate([cosv, sinv], axis=1)),
            "tabTT": _bf(np.concatenate([cosvT, sinvT], axis=1)),
            "bqm": np.ascontiguousarray(
                np.concatenate([bqes_img, m3], axis=1)),
            "bkvid": _bf(bkvid_img),
        })

    if "a" not in _CACHE:
        _CACHE["a"] = build_kernel_a()
    nca = _CACHE["a"]
    if not getattr(nca, "_wsplit_done", False):
        _split_multi_waits(nca)
        nca._wsplit_done = True
    res_a = run_bass_kernel_spmd(nca, in_maps_a, list(range(NC))).results

    # reshard: core d of kernel B gets flat tokens [d*512, (d+1)*512).
    # srmsnorm and the gate fold into the reshard (elementwise glue):
    #   z = (u + bu) * y * rsqrt(mean(y^2) + eps), split into fp8 hi/lo
    NT = B * N // NC
    ys = np.stack([np.asarray(res_a[h]["y"]).astype(np.float32)
                   for h in range(H)])  # [H, B, 128, N]
    us = np.stack([np.asarray(res_a[h]["u"]).astype(np.float32)
                   for h in range(H)])  # [H, B, 128, N]
    ms = np.einsum("hbdn,hbdn->bn", ys, ys) * (1.0 / D)
    rs = 1.0 / np.sqrt(ms + EPS)  # [B, N]
    bu_f = qkvu_b[3 * D:4 * D].reshape(H, 128)  # [h, p]
    # z in [H, B, 128, N] head-parallel layout
    zf = (us + bu_f[:, None, :, None]) * ys * rs[None, :, None, :]
    wo_img = out_w.reshape(8, 128, 8, 128).transpose(3, 0, 2, 1).reshape(
        128, 64 * 128)  # [p, (oc*8+e)*128+o] = wo[oc*128+o, e*128+p]
    wo8, dwo8 = _f8split(wo_img * WS)

    in_maps_b = []
    for d in range(NC):
        bb, off = d // 4, (d % 4) * NT
        # z image [p, e*NT + t] = z[token off+t, e*128+p] = zf[e, bb, p, ...]
        z_img = np.ascontiguousarray(
            zf[:, bb, :, off:off + NT].transpose(1, 0, 2).reshape(128, 8 * NT))
        z8, dz8 = _f8split(z_img)
        in_maps_b.append({
            "z8": z8,
            "dz8": dz8,
            "wo8": wo8,
            "dwo8": dwo8,
        })

    if "b" not in _CACHE:
        _CACHE["b"] = build_kernel_b()
    ncb = _CACHE["b"]
    if not getattr(ncb, "_wsplit_done", False):
        _split_multi_waits(ncb)
        ncb._wsplit_done = True
    res_b = run_bass_kernel_spmd(ncb, in_maps_b, list(range(NC))).results

    out = np.empty((B * N, D), np.float32)
    for d in range(NC):
        o = np.asarray(res_b[d]["outT"]).astype(np.float32) * (1.0 / WS)
        out[d * NT:(d + 1) * NT] = o.reshape(128, 8, NT).transpose(
            2, 1, 0).reshape(NT, D)
    out += out_b[None, :]
    return out.reshape(B, N, D)


# revision 127
# speedup vs baseline: 1.0030x; 1.0030x over previous
"""NormLinearAttention kernel for 8 Trainium2 NeuronCores (fp8 build).

Strategy:
  Kernel A: head-parallel (core c owns head c, both batches): q/k/v AND
    the u gate projection for the head's channel slice, plus the full
    linear-attention pipeline. Projections use compensated fp8 DoubleRow
    matmuls: operands split host-side into fp8 hi + lo residual
    (x = x8 + dx8, 64w = w8 + dw8); each 1024-contraction is 12 DoubleRow
    matmuls (x8w8 + dw8x8 + w8dx8, 4 e-pairs each, K=256/instr at 0.5
    cyc/row) -> 25% fewer PE cycles than bf16 at ~bf16 accuracy. The /64
    weight prescale folds into the silu/copy activation scales.
    Linear attention with mask = exp(s*tril):
        out_i = (e^s-1)*causal_i + full_i
    chunked at 256 (pairs of 128 q-blocks); the running state P
    accumulates in one persistent PSUM bank (start=False chains) shared
    by both batches, and Q_m = (e^s-1)*P_m is snapshotted via Act copies
    with the per-core es1 scale.
  Host reshard (elementwise glue only): computes the SRMSNorm and gate
    fold z = (u + bu) * y * rsqrt(mean(y^2)+eps), splits z into fp8
    hi/lo, and re-tiles token-parallel.
  Kernel B: token-parallel (512 tokens/core) out-projection only, 8
    interleaved open PSUM groups whose terms are emitted in operand
    stream-arrival order; stage copies alternate Act/DVE.

Inputs are packed into few large DMAs (each costs ~625ns of shared
HWDGE); x streams as x8(t+1)-ahead-of-dx8(t) so two thirds of every
projection group can run before its residual half lands.
"""

import sys

sys.path.insert(0, "/opt/trn_rl_repo")

import numpy as np
import ml_dtypes

import bass_rust
import concourse.bass as bass
import concourse.mybir as mybir
import concourse.tile as tile
from concourse.bass_utils import run_bass_kernel_spmd

F32 = mybir.dt.float32
BF16 = mybir.dt.bfloat16
FP8 = mybir.dt.float8e4
AF = mybir.ActivationFunctionType
ALU = mybir.AluOpType
DR = mybir.MatmulPerfMode.DoubleRow

B, N, D = 2, 2048, 1024
H = 8
HD = 128
NC = 8
TT = 512           # projection token tile
NB = N // 128      # 16 token blocks per batch
NS = N // 256      # 8 state chunks per batch
EPS = 1e-6
WS = 64.0          # fp8 weight prescale
BF_NP = ml_dtypes.bfloat16
F8_NP = ml_dtypes.float8_e4m3


def _split_multi_waits(nc, max_waits=1):
    """walrus accepts one sync wait per instruction; Tile emits several.
    Insert same-engine NoOps each carrying one of the extra waits."""
    ctr = 0
    for func in nc.m.functions:
        for blk in func.blocks:
            out = []
            changed = False
            for inst in blk.instructions:
                si = inst.sync_info
                if si is not None and len(si.on_wait) > max_waits:
                    waits = list(si.on_wait)
                    for w in waits[:-max_waits]:
                        nop = bass_rust.InstNoOp(name=f"WSPLIT-{ctr}", ins=[], outs=[])
                        ctr += 1
                        nop.engine = inst.engine
                        nop.sync_info = mybir.SyncInfo(on_wait=[w], on_update=[])
                        out.append(nop)
                    inst.sync_info = mybir.SyncInfo(
                        on_wait=waits[-max_waits:], on_update=list(si.on_update)
                    )
                    changed = True
                out.append(inst)
            if changed:
                blk.instructions = out
    return ctr


def build_kernel_a(dbg=False):
    nc = bass.Bass("TRN2", target_bir_lowering=False, debug=False, num_devices=NC)
    # packed inputs: every extra DMA costs ~625ns on the shared HWDGE, so
    # tensors are fused host-side into few large transfers
    xpk_in = nc.dram_tensor("xpk", [B, 4, 128, 2 * 8 * TT], FP8,
                            kind="ExternalInput").ap()   # x8 || dx8 per tile
    wpk_in = nc.dram_tensor("wpk", [128, 8192], FP8,
                            kind="ExternalInput").ap()   # wq8|wkv8|wu8|d...
    tabT_in = nc.dram_tensor("tabT", [128, 2 * N], BF16,
                             kind="ExternalInput").ap()  # cosT || sinT
    tabTT_in = nc.dram_tensor("tabTT", [128, 2 * N], BF16,
                              kind="ExternalInput").ap()  # cosTT || sinTT
    bqm_in = nc.dram_tensor("bqm", [128, 2 + 384], F32,
                            kind="ExternalInput").ap()   # bqes || mask3
    bkvid_in = nc.dram_tensor("bkvid", [128, 640], BF16, kind="ExternalInput").ap()
    y_out = nc.dram_tensor("y", [B, 128, N], BF16, kind="ExternalOutput").ap()
    u_out = nc.dram_tensor("u", [B, 128, N], BF16, kind="ExternalOutput").ap()
    dbg_outs = {}
    if dbg:
        for nm in ("qec", "qes", "kec", "kes", "knc", "kns", "ksn", "ksT"):
            dbg_outs[nm] = nc.dram_tensor(
                "d_" + nm, [128, N], BF16, kind="ExternalOutput").ap()
        dbg_outs["kvnat"] = nc.dram_tensor(
            "d_kvnat", [128, NB * 256], BF16, kind="ExternalOutput").ap()
        dbg_outs["QB"] = nc.dram_tensor(
            "d_QB", [128, NS * 256], BF16, kind="ExternalOutput").ap()

    with tile.TileContext(nc) as tc:
        with tc.tile_pool(name="consts", bufs=1) as cp, \
             tc.tile_pool(name="big", bufs=1) as bp, \
             tc.tile_pool(name="xt", bufs=7) as xp, \
             tc.tile_pool(name="qsil", bufs=3) as qsp, \
             tc.tile_pool(name="em", bufs=4) as emp:
            # wpk first half (wq+wkv hi), then the first x tile split so the
            # first projection matmuls start as early as possible
            xt00 = xp.tile([128, 2 * 8 * TT], FP8, tag="xt")
            wpk = cp.tile([128, 8192], FP8)
            tabT = cp.tile([128, 2 * N], BF16)
            tabTT = cp.tile([128, 2 * N], BF16)
            bqm = cp.tile([128, 2 + 384], F32)
            bkvid = cp.tile([128, 640], BF16)
            bz = cp.tile([128, 1], F32)
            wqt = wpk[:, 0:1024]
            wkvt = wpk[:, 1024:3072]
            wut = wpk[:, 3072:4096]
            dwqt = wpk[:, 4096:5120]
            dwkvt = wpk[:, 5120:7168]
            dwut = wpk[:, 7168:8192]
            cosT = tabT[:, 0:N]
            sinT = tabT[:, N:2 * N]
            cosTT = tabTT[:, 0:N]
            sinTT = tabTT[:, N:2 * N]
            bq = bqm[:, 0:1]
            es1 = bqm[:, 1:2]
            mask3 = bqm[:, 2:386]
            bkv = bkvid[:, 0:512]
            ident = bkvid[:, 512:640]
            # tiny bias/scale tensors first: the first silu and the first kv
            # consumer block on them, so they must not queue behind the bulk
            nc.sync.dma_start(bqm[:], bqm_in)
            nc.sync.dma_start(wpk[:, 0:4096], wpk_in[:, 0:4096])
            nc.sync.dma_start(bkvid[:], bkvid_in)
            nc.sync.dma_start(xt00[:, 0:4 * TT], xpk_in[0][0][:, 0:4 * TT])
            nc.sync.dma_start(xt00[:, 4 * TT:8 * TT], xpk_in[0][0][:, 4 * TT:8 * TT])
            nc.sync.dma_start(wpk[:, 4096:8192], wpk_in[:, 4096:8192])
            nc.vector.memset(bz[:], 0.0)
            # warm the PE p-state ramp while the first DMAs land: throwaway
            # matmuls on a zeroed tile so real matmuls start at full speed
            wz = qsp.tile([128, TT], BF16, tag="warm")
            nc.vector.memset(wz[:], 0.0)

            st = []  # per-batch persistent tensors
            for b in range(B):
                d = {}
                for nm, shape, dt in (
                    ("kvnat", [128, NB * 256], BF16),
                    ("ksn", [128, N], BF16),
                    ("ksT", [128, N], BF16),
                    ("qec", [128, N], BF16),
                    ("qes", [128, N], BF16),
                    ("kec", [128, N], BF16),
                    ("kes", [128, N], BF16),
                    ("knc", [128, N], BF16),
                    ("kns", [128, N], BF16),
                    ("QB", [128, NS * 256], BF16),
                    ("yst", [128, N], BF16),
                    ("ust", [128, N], BF16),
                ):
                    d[nm] = bp.tile(shape, dt, tag=f"{nm}{b}", name=f"{nm}{b}")
                st.append(d)

            def vn_blk(b, j):
                # v for token block j lives in the kv-paired tile
                return st[b]["kvnat"][:, j * 256 + 128:(j + 1) * 256]

            def phase1_tile0(pq, pkv, ptp, pup, xt):
                """Tile (b=0, t=0) with stage-interleaved emission: all six
                PSUM groups (q, 4 kv halves, u) advance term-by-term so the
                in-order PE queue runs every matmul whose operands have
                landed, instead of parking a group on a late DMA."""
                b, t = 0, 0
                s = st[0]
                tsl = slice(0, TT)
                xk = xt.rearrange("p (k e t) -> p k e t", k=2, e=8)
                wqr = wqt.rearrange("p (e o) -> p e o", e=8)
                dwqr = dwqt.rearrange("p (e o) -> p e o", e=8)
                wur = wut.rearrange("p (e o) -> p e o", e=8)
                dwur = dwut.rearrange("p (e o) -> p e o", e=8)
                wkvr = wkvt.rearrange("p (e kv) -> p e kv", e=8)
                dwkvr = dwkvt.rearrange("p (e kv) -> p e kv", e=8)
                psq = pq.tile([128, TT], F32, tag="psq")
                pskvs = [pkv.tile([128, 512], F32, tag="pskv", name=f"pskv0{j}")
                         for j in range(2)]
                psu = pup.tile([128, TT], F32, tag="pu")

                def em_hd(ps, wr_, kk, first, last):
                    for ep in range(4):
                        pe = slice(2 * ep, 2 * ep + 2)
                        nc.tensor.matmul(
                            ps[:], wr_[:, pe, :], xk[:, kk, pe, :],
                            start=(first and ep == 0), stop=(last and ep == 3),
                            perf_mode=DR)

                def em_kv(jb, ws_, kk, first, last):
                    pskv = pskvs[jb // 2]
                    osl = slice((jb % 2) * 256, (jb % 2 + 1) * 256)
                    tok = slice(jb * 128, (jb + 1) * 128)
                    for ep in range(4):
                        pe = slice(2 * ep, 2 * ep + 2)
                        nc.tensor.matmul(
                            pskv[:, osl], xk[:, kk, pe, tok], ws_[:, pe, :],
                            # only the bank's first half opens with start=True:
                            # the bank-level zero covers the second half
                            start=(first and ep == 0 and jb % 2 == 0),
                            stop=(last and ep == 3), perf_mode=DR)

                for si, (dwf, kk) in enumerate(((0, 0), (1, 0), (0, 1))):
                    first, last = si == 0, si == 2
                    em_hd(psq, dwqr if dwf else wqr, kk, first, last)
                    for jb in range(4):
                        em_kv(jb, dwkvr if dwf else wkvr, kk, first, last)
                    em_hd(psu, dwur if dwf else wur, kk, first, last)

                qsil = qsp.tile([128, TT], BF16, tag="qsil")
                nc.scalar.activation(qsil[:], psq[:], AF.Silu, bias=bq,
                                     scale=1.0 / WS)
                ptr4 = ptp.tile([128, 512], BF16, tag="ptr4")
                for jp in range(2):
                    jj0 = jp * 2
                    nc.vector.scalar_tensor_tensor(
                        s["kvnat"][:, jj0 * 256:(jj0 + 2) * 256], pskvs[jp][:],
                        1.0 / WS, bkv, ALU.mult, ALU.add)
                    kpair = s["kvnat"].rearrange("p (j kv c) -> p j kv c",
                                                 kv=2, c=128)
                    nc.scalar.activation(
                        s["ksn"][:, jj0 * 128:(jj0 + 2) * 128],
                        kpair[:, jj0:jj0 + 2, 0],
                        AF.Silu, bias=bz[:])
                    hsl = slice(jp * 256, (jp + 1) * 256)
                    nc.vector.tensor_mul(s["knc"][:, hsl], s["ksn"][:, hsl],
                                         cosTT[:, hsl])
                    nc.vector.tensor_mul(s["kns"][:, hsl], s["ksn"][:, hsl],
                                         sinTT[:, hsl])
                    for half in range(2):
                        jb = jp * 2 + half
                        nc.tensor.transpose(ptr4[:, jb * 128:(jb + 1) * 128],
                                            s["ksn"][:, jb * 128:(jb + 1) * 128],
                                            ident)
                nc.scalar.copy(s["ksT"][:, tsl], ptr4[:])
                nc.scalar.activation(s["ust"][:, tsl], psu[:], AF.Copy,
                                     scale=1.0 / WS)
                nc.vector.tensor_mul(s["kec"][:, tsl], s["ksT"][:, tsl], cosT[:, tsl])
                nc.vector.tensor_mul(s["kes"][:, tsl], s["ksT"][:, tsl], sinT[:, tsl])
                nc.vector.tensor_mul(s["qec"][:, tsl], qsil[:], cosT[:, tsl])
                nc.vector.tensor_mul(s["qes"][:, tsl], qsil[:], sinT[:, tsl])

            def phase1_tile(b, t, pq, pkv, ptp, pup, xt):
                s = st[b]
                tsl = slice(t * TT, (t + 1) * TT)
                xk = xt.rearrange("p (k e t) -> p k e t", k=2, e=8)
                wqr = wqt.rearrange("p (e o) -> p e o", e=8)
                dwqr = dwqt.rearrange("p (e o) -> p e o", e=8)
                wur = wut.rearrange("p (e o) -> p e o", e=8)
                dwur = dwut.rearrange("p (e o) -> p e o", e=8)
                wkvr = wkvt.rearrange("p (e kv) -> p e kv", e=8)
                dwkvr = dwkvt.rearrange("p (e kv) -> p e kv", e=8)
                # q projection: [hd, tok], compensated fp8 DoubleRow.
                # Term order (w8*x8, dw8*x8, w8*dx8): the first two-thirds of
                # every group need only the x8 half of the tile, which the
                # DMA stream sends one tile ahead of the dx8 halves.
                psq = pq.tile([128, TT], F32, tag="psq")
                ti = 0
                for wr_, kk in ((wqr, 0), (dwqr, 0), (wqr, 1)):
                    for ep in range(4):
                        pe = slice(2 * ep, 2 * ep + 2)
                        nc.tensor.matmul(
                            psq[:], wr_[:, pe, :], xk[:, kk, pe, :],
                            start=(ti == 0), stop=(ti == 11), perf_mode=DR)
                        ti += 1
                qsil = qsp.tile([128, TT], BF16, tag="qsil" if b == 0 else f"qsil1_{t}",
                                bufs=1 if b else None)
                nc.scalar.activation(qsil[:], psq[:], AF.Silu, bias=bq, scale=1.0 / WS)
                # k|v paired projection: [tok, k|v]; two token blocks share a
                # psum bank as sequential accumulation groups
                ptr4 = ptp.tile([128, 512], BF16, tag="ptr4")
                for jp in range(2):
                    pskv = pkv.tile([128, 512], F32, tag="pskv")
                    for half in range(2):
                        jb = jp * 2 + half
                        osl = slice(half * 256, (half + 1) * 256)
                        tok = slice(jb * 128, (jb + 1) * 128)
                        ti = 0
                        for kk, ws_ in ((0, wkvr), (0, dwkvr), (1, wkvr)):
                            for ep in range(4):
                                pe = slice(2 * ep, 2 * ep + 2)
                                nc.tensor.matmul(
                                    pskv[:, osl], xk[:, kk, pe, tok], ws_[:, pe, :],
                                    start=(ti == 0), stop=(ti == 11), perf_mode=DR)
                                ti += 1
                    jj0 = t * 4 + jp * 2
                    nc.vector.scalar_tensor_tensor(
                        s["kvnat"][:, jj0 * 256:(jj0 + 2) * 256], pskv[:],
                        1.0 / WS, bkv, ALU.mult, ALU.add)
                    kpair = s["kvnat"].rearrange("p (j kv c) -> p j kv c",
                                                 kv=2, c=128)
                    nc.scalar.activation(
                        s["ksn"][:, jj0 * 128:(jj0 + 2) * 128],
                        kpair[:, jj0:jj0 + 2, 0],
                        AF.Silu, bias=bz[:])
                    # state-path lrpe for this half right after its silu:
                    # the interleaved 2a chunk matmuls wait on these
                    hsl = slice(t * TT + jp * 256, t * TT + (jp + 1) * 256)
                    nc.vector.tensor_mul(s["knc"][:, hsl], s["ksn"][:, hsl],
                                         cosTT[:, hsl])
                    nc.vector.tensor_mul(s["kns"][:, hsl], s["ksn"][:, hsl],
                                         sinTT[:, hsl])
                    for half in range(2):
                        jb = jp * 2 + half
                        jj = t * 4 + jb
                        nc.tensor.transpose(ptr4[:, jb * 128:(jb + 1) * 128],
                                            s["ksn"][:, jj * 128:(jj + 1) * 128],
                                            ident)
                nc.scalar.copy(s["ksT"][:, tsl], ptr4[:])
                # u projection for this head's channel slice: feeds the host
                # z-fold between launches. Emitted after kv so the 2a chain's
                # inputs (knc/kns) are produced while u's matmuls run.
                psu = pup.tile([128, TT], F32, tag="pu")
                ti = 0
                for wr_, kk in ((wur, 0), (dwur, 0), (wur, 1)):
                    for ep in range(4):
                        pe = slice(2 * ep, 2 * ep + 2)
                        nc.tensor.matmul(
                            psu[:], wr_[:, pe, :], xk[:, kk, pe, :],
                            start=(ti == 0), stop=(ti == 11), perf_mode=DR)
                        ti += 1
                nc.scalar.activation(s["ust"][:, tsl], psu[:], AF.Copy,
                                     scale=1.0 / WS)
                if t % 2 == 1:
                    gsl = slice((t - 1) * TT, (t + 1) * TT)
                    nc.sync.dma_start(u_out[b][:, gsl], s["ust"][:, gsl])
                # q/kec/kes are only read in 2b.  For b=1 those muls are
                # deferred so they don't sit in the DVE queue ahead of
                # 2b(0)'s masked multiplies.

                def lrpe_2b():
                    nc.vector.tensor_mul(s["kec"][:, tsl], s["ksT"][:, tsl], cosT[:, tsl])
                    nc.vector.tensor_mul(s["kes"][:, tsl], s["ksT"][:, tsl], sinT[:, tsl])
                    nc.vector.tensor_mul(s["qec"][:, tsl], qsil[:], cosT[:, tsl])
                    nc.vector.tensor_mul(s["qes"][:, tsl], qsil[:], sinT[:, tsl])
                if b == 0:
                    lrpe_2b()
                    return None
                return lrpe_2b

            def phase2a_chunk(b, ms, stp):
                s = st[b]
                if ms > 0:
                    # snapshot Q_ms = (e^s-1)*P_ms before adding chunk ms
                    nc.scalar.activation(s["QB"][:, ms * 256:(ms + 1) * 256],
                                         stp[:], AF.Copy, scale=es1)
                # running state accumulates in the persistent PSUM bank.
                # start=True only on the very first matmul: it zeroes the
                # whole 2KB bank (both g halves), so g=1's first write and
                # every later chunk must accumulate (start=False) or the
                # bank-level pending-zero wipes the other half's history.
                for g in range(2):
                    kn = s["knc"] if g == 0 else s["kns"]
                    for s2 in range(2):
                        j = 2 * ms + s2
                        bsl = slice(j * 128, (j + 1) * 128)
                        nc.tensor.matmul(stp[:, g * 128:(g + 1) * 128],
                                         kn[:, bsl], vn_blk(b, j),
                                         start=(ms == 0 and s2 == 0 and g == 0),
                                         stop=(s2 == 1),
                                         skip_group_check=True)

            def phase2a_post(b, stp):
                s = st[b]
                # Q_0 = S_full; Q_m += S_full. The adds are independent and
                # bf16/SBUF, so DVE's 2x mode (194ns) beats Pool (603ns);
                # split across both so neither chain gates 2b's outputs.
                nc.scalar.activation(s["QB"][:, 0:256], stp[:], AF.Copy)
                for ms in range(1, NS):
                    msl = slice(ms * 256, (ms + 1) * 256)
                    nc.gpsimd.tensor_add(s["QB"][:, msl], s["QB"][:, msl],
                                         s["QB"][:, 0:256])

            def phase2b(b, pet, pot, interleave=None):
                s = st[b]

                def emit_et(ms):
                    etb = pet.tile([128, 512], F32, tag="etb", name=f"etb{ms}")
                    for s2 in range(2):
                        qb = 2 * ms + s2
                        qsl = slice(qb * 128, (qb + 1) * 128)
                        et = etb[:, s2 * 128:(s2 + 1) * 128]
                        nc.tensor.matmul(et, s["kec"][:, qsl], s["qec"][:, qsl],
                                         start=True, stop=False)
                        nc.tensor.matmul(et, s["kes"][:, qsl], s["qes"][:, qsl],
                                         start=False, stop=True)
                    bsl = slice(2 * ms * 128, (2 * ms + 1) * 128)
                    q1sl = slice((2 * ms + 1) * 128, (2 * ms + 2) * 128)
                    et2 = etb[:, 256:384]
                    nc.tensor.matmul(et2, s["kec"][:, bsl], s["qec"][:, q1sl],
                                     start=True, stop=False)
                    nc.tensor.matmul(et2, s["kes"][:, bsl], s["qes"][:, q1sl],
                                     start=False, stop=True)
                    # one masked multiply for all three energy blocks
                    em3 = emp.tile([128, 384], BF16, tag="em", name=f"em_{ms}")
                    nc.vector.tensor_mul(em3[:], etb[:, 0:384], mask3[:])
                    return em3

                def emit_ot(ms, em3):
                    qmsl = slice(ms * 256, ms * 256 + 128)
                    smsl = slice(ms * 256 + 128, (ms + 1) * 256)
                    otb = pot.tile([128, 256], F32, tag="otb", name=f"otb{ms}")
                    for s2 in range(2):
                        qb = 2 * ms + s2
                        qsl = slice(qb * 128, (qb + 1) * 128)
                        ot = otb[:, s2 * 128:(s2 + 1) * 128]
                        nc.tensor.matmul(ot, s["QB"][:, qmsl], s["qec"][:, qsl],
                                         start=True, stop=False)
                        nc.tensor.matmul(ot, s["QB"][:, smsl], s["qes"][:, qsl],
                                         start=False, stop=False)
                        if s2 == 1:
                            nc.tensor.matmul(ot, vn_blk(b, 2 * ms),
                                             em3[:, 256:384],
                                             start=False, stop=False)
                        nc.tensor.matmul(ot, vn_blk(b, qb),
                                         em3[:, s2 * 128:(s2 + 1) * 128],
                                         start=False, stop=True)
                    # stage via SBUF (DGE cannot read PSUM); ship pairs,
                    # the final ones singly to shorten the drain tail
                    osl = slice(2 * ms * 128, (2 * ms + 2) * 128)
                    nc.scalar.copy(s["yst"][:, osl], otb[:])
                    if ms % 2 == 1:
                        gsl = slice((ms // 2) * 512, (ms // 2 + 1) * 512)
                        nc.sync.dma_start(y_out[b][:, gsl], s["yst"][:, gsl])

                # software pipeline: energies run ahead of outputs; other
                # batches' deferred DVE work interleaves under the PE matmuls
                DEPTH = 2
                ems = {}
                for ms in range(NS):
                    ems[ms] = emit_et(ms)
                    if interleave and ms % 2 == 1:
                        interleave[ms // 2]()
                    if ms >= DEPTH:
                        emit_ot(ms - DEPTH, ems.pop(ms - DEPTH))
                for ms in range(NS - DEPTH, NS):
                    emit_ot(ms, ems.pop(ms))

            with tc.tile_pool(name="pq", bufs=2, space="PSUM") as pq, \
                 tc.tile_pool(name="pkv", bufs=3, space="PSUM") as pkv, \
                 tc.tile_pool(name="ptr", bufs=1, space="PSUM") as ptp, \
                 tc.tile_pool(name="pu", bufs=1, space="PSUM") as pup, \
                 tc.tile_pool(name="pst", bufs=1, space="PSUM") as pstp:
                # one state bank shared by both batches: b1's first matmul
                # (start=True) re-zeroes it after b0's snapshots are taken
                stp0 = pstp.tile([128, 256], F32, tag="stp", name="stp")
                stp = [stp0, stp0]
                # prefetch x in x8/dx8-interleaved order: tile t's first 8
                # matmuls per group need only the x8 half, so stream
                # x8(t+1) ahead of dx8(t); tables slot into the gaps by
                # first-use time (tabTT at tile0's kv, tabT by 2b)
                allx = [xt00]
                for i in range(1, 8):
                    xtn = xp.tile([128, 2 * 8 * TT], FP8, tag="xt", name=f"xt{i}")
                    allx.append(xtn)
                xts0, xts1 = allx[0:4], allx[4:8]
                srcs = [xpk_in[i // 4][i % 4] for i in range(8)]
                # prefetch x in x8/dx8-interleaved order: tile t's first 8
                # matmuls per group need only the x8 half, so stream
                # x8(t+1) ahead of dx8(t); tables slot into the gaps by
                # first-use time (tabTT at tile0's kv, tabT by 2b)
                nc.sync.dma_start(allx[1][:, 0:8 * TT], srcs[1][:, 0:8 * TT])
                nc.sync.dma_start(xt00[:, 8 * TT:16 * TT],
                                  srcs[0][:, 8 * TT:16 * TT])
                nc.sync.dma_start(tabTT[:], tabTT_in)
                for i in range(2, 8):
                    nc.sync.dma_start(allx[i][:, 0:8 * TT], srcs[i][:, 0:8 * TT])
                    nc.sync.dma_start(allx[i - 1][:, 8 * TT:16 * TT],
                                      srcs[i - 1][:, 8 * TT:16 * TT])
                    if i == 2:
                        nc.sync.dma_start(tabT[:], tabT_in)
                nc.sync.dma_start(allx[7][:, 8 * TT:16 * TT],
                                  srcs[7][:, 8 * TT:16 * TT])
                wp = pq.tile([128, TT], F32, tag="psq", name="warmps")
                for i in range(8):
                    nc.tensor.matmul(wp[:], wz[:, 0:128], wz[:],
                                     start=True, stop=True)
                # state chunks interleave with projection tiles, lagging one
                # chunk behind their producers so the Act snapshot + DVE lrpe
                # chain never stalls the in-order PE queue
                # state chunks lag their producer tile by one chunk so the
                # Act snapshot + DVE lrpe chain never stalls the in-order PE
                for t in range(4):
                    phase1_tile(0, t, pq, pkv, ptp, pup, xts0[t])
                    if t > 0:
                        phase2a_chunk(0, 2 * t - 1, stp[0])
                    phase2a_chunk(0, 2 * t, stp[0])
                phase2a_chunk(0, 7, stp[0])
                phase2a_post(0, stp[0])
                deferred = []
                for t in range(4):
                    deferred.append(
                        phase1_tile(1, t, pq, pkv, ptp, pup, xts1[t]))
                    if t > 0:
                        phase2a_chunk(1, 2 * t - 1, stp[1])
                    phase2a_chunk(1, 2 * t, stp[1])
                phase2a_chunk(1, 7, stp[1])
                phase2a_post(1, stp[1])
            with tc.tile_pool(name="pet", bufs=3, space="PSUM") as pet, \
                 tc.tile_pool(name="pot", bufs=3, space="PSUM") as pot:
                # b1's 2b-only lrpe muls run on DVE while 2b(0) computes
                phase2b(0, pet, pot)
                for fn in deferred:
                    fn()
                phase2b(1, pet, pot)
            if dbg:
                for nm, dst in dbg_outs.items():
                    nc.sync.dma_start(dst, st[0][nm][:])

    return nc


def build_kernel_b():
    """o-projection only: z = (u+bu)*yhat is folded host-side between the
    launches (elementwise glue on the reshard path) and arrives pre-split
    into fp8 hi/lo. out_t = z @ (WS*wo).T is shipped as WS*(out - out_b)."""
    nc = bass.Bass("TRN2", target_bir_lowering=False, debug=False, num_devices=NC)
    NT = B * N // NC  # 512 tokens per core
    z8_in = nc.dram_tensor("z8", [128, 8 * NT], FP8, kind="ExternalInput").ap()
    dz8_in = nc.dram_tensor("dz8", [128, 8 * NT], FP8, kind="ExternalInput").ap()
    wo8_in = nc.dram_tensor("wo8", [128, 64 * 128], FP8, kind="ExternalInput").ap()
    dwo8_in = nc.dram_tensor("dwo8", [128, 64 * 128], FP8,
                             kind="ExternalInput").ap()
    out_t = nc.dram_tensor("outT", [128, 8 * NT], BF16, kind="ExternalOutput").ap()

    with tile.TileContext(nc) as tc:
        with tc.tile_pool(name="ins", bufs=1) as ip, \
             tc.tile_pool(name="work", bufs=1) as wk, \
             tc.tile_pool(name="po", bufs=8, space="PSUM") as pop:
            z8 = ip.tile([128, 8 * NT], FP8)
            dz8 = ip.tile([128, 8 * NT], FP8)
            wo8 = ip.tile([128, 64 * 128], FP8)
            dwo8 = ip.tile([128, 64 * 128], FP8)
            # stream operands in first-use order of the staged group emission
            nc.sync.dma_start(z8[:], z8_in)
            nc.sync.dma_start(wo8[:, 0:1024], wo8_in[:, 0:1024])
            nc.sync.dma_start(wo8[:, 1024:2048], wo8_in[:, 1024:2048])
            nc.sync.dma_start(wo8[:, 2048:4096], wo8_in[:, 2048:4096])
            nc.sync.dma_start(dz8[:], dz8_in)
            nc.sync.dma_start(wo8[:, 4096:8192], wo8_in[:, 4096:8192])
            nc.sync.dma_start(dwo8[:, 0:4096], dwo8_in[:, 0:4096])
            nc.sync.dma_start(dwo8[:, 4096:8192], dwo8_in[:, 4096:8192])

            # warm the PE p-state ramp during the DMA lead-in (rotates into
            # the po pool: its bank is recycled by the 8th o-group). Sized to
            # end right as z8+wo8h1 land: each costs ~213ns at mid p-state,
            # and 16 of them cover the ~3.4us lead-in exactly.
            wz = wk.tile([128, NT], BF16, tag="warm")
            nc.vector.memset(wz[:], 0.0)
            wp = pop.tile([128, NT], F32, tag="po", name="warmps")
            for i in range(15):
                nc.tensor.matmul(wp[:, 0:NT // 2], wz[:, 0:128],
                                 wz[:, 0:NT // 2], start=True, stop=True)

            ostage = wk.tile([128, 8 * NT], BF16, tag="ostage")
            z8r = z8.rearrange("p (e t) -> p e t", e=8)
            dz8r = dz8.rearrange("p (e t) -> p e t", e=8)
            wo8r = wo8.rearrange("p (u e o) -> p u e o", u=8, e=8)
            dwo8r = dwo8.rearrange("p (u e o) -> p u e o", u=8, e=8)

            # all 8 PSUM groups stay open; terms are emitted globally in the
            # order their operands stream in, so the in-order PE queue never
            # parks behind a late tensor
            pss = [pop.tile([128, NT], F32, tag="po", name=f"po{oc}")
                   for oc in range(8)]

            def o_term(ocs, ws_, zs_, first=False, last=False,
                       ship_at=(1, 3, 5, 7)):
                for oc in ocs:
                    ps = pss[oc]
                    for ep in range(4):
                        pe = slice(2 * ep, 2 * ep + 2)
                        nc.tensor.matmul(
                            ps[:], ws_[:, oc, pe, :], zs_[:, pe, :],
                            start=(first and ep == 0), stop=(last and ep == 3),
                            perf_mode=DR)
                    if last:
                        # stage copies alternate Act/DVE so the two chains
                        # drain in parallel instead of serializing on Act
                        osl = slice(oc * NT, (oc + 1) * NT)
                        if oc % 2 == 0:
                            nc.scalar.copy(ostage[:, osl], ps[:])
                        else:
                            nc.vector.tensor_scalar_mul(ostage[:, osl],
                                                        ps[:], 1.0)
                        # early blocks ship in pairs (each DMA holds the
                        # shared HWDGE ~625ns, so fewer slots ahead of the
                        # final ship); the last two ship singly
                        if oc in ship_at:
                            g0 = (oc // 2) * 2
                            gsl = slice(g0 * NT, (g0 + 2) * NT)
                            nc.sync.dma_start(out_t[:, gsl], ostage[:, gsl])

            q0, q1, q2 = range(0, 2), range(2, 4), range(4, 8)
            o_term(q0, wo8r, z8r, first=True)   # needs z8 + wo8 q1
            o_term(q1, wo8r, z8r, first=True)   # + wo8 q2
            o_term(q0, wo8r, dz8r)              # + dz8
            o_term(q1, wo8r, dz8r)
            o_term(q2, wo8r, z8r, first=True)   # + wo8 h2
            o_term(q0, dwo8r, z8r, last=True)   # + dwo8 h1
            o_term(q1, dwo8r, z8r, last=True)
            o_term(q2, wo8r, dz8r)
            o_term(q2, dwo8r, z8r, last=True)   # + dwo8 h2

    return nc


_CACHE = {}


def _bf(a):
    return np.ascontiguousarray(a.astype(BF_NP))


def _f8split(a):
    """fp8 hi + fp8 residual decomposition of a float32 array."""
    hi = a.astype(F8_NP)
    lo = (a - hi.astype(np.float32)).astype(F8_NP)
    return np.ascontiguousarray(hi), np.ascontiguousarray(lo)


def kernel(x, slope_rate, qkvu_w, qkvu_b, out_w, out_b, theta):
    x = np.asarray(x, np.float32)
    slope_rate = np.asarray(slope_rate, np.float32)
    qkvu_w = np.asarray(qkvu_w, np.float32)
    qkvu_b = np.asarray(qkvu_b, np.float32)
    out_w = np.asarray(out_w, np.float32)
    out_b = np.asarray(out_b, np.float32)
    theta = np.asarray(theta, np.float32)

    # x in [b, t, p, e*512+j] layout: xh[b,t,p,e*512+j] = x[b, t*512+j, e*128+p]
    xh = (x.reshape(B, 4, TT, 8, 128).transpose(0, 1, 4, 3, 2)
          .reshape(B, 4, 128, 8 * TT))
    x8h, dx8h = _f8split(xh)
    xpk = np.ascontiguousarray(np.concatenate([x8h, dx8h], axis=-1))
    idx = np.arange(N, dtype=np.float32)
    ident = np.eye(128, dtype=np.float32)

    in_maps_a = []
    for c in range(NC):
        th = theta[c, 0].astype(np.float32)[:, None] * idx[None, :]  # [128, N]
        es = np.exp(slope_rate[c, 0, 0]).astype(np.float32)
        es1 = np.float32(es - 1.0)
        sl = slice(c * HD, (c + 1) * HD)
        wq = qkvu_w[0 * D:1 * D][sl]   # [128, D]
        wk = qkvu_w[1 * D:2 * D][sl]
        wv = qkvu_w[2 * D:3 * D][sl]
        # wq image: [p, e*128+o] = wq[o, e*128+p]
        wq_img = wq.T.reshape(8, 128, 128).transpose(1, 0, 2).reshape(128, 8 * 128)
        # wkv image: [p, e*256 + (k|v 128+o)]
        wkv_img = np.concatenate(
            [wk.T.reshape(8, 128, 1, 128), wv.T.reshape(8, 128, 1, 128)], axis=2
        ).transpose(1, 0, 2, 3).reshape(128, 8 * 256)
        wu_h = qkvu_w[3 * D:4 * D][sl]
        wu_img = wu_h.T.reshape(8, 128, 128).transpose(1, 0, 2).reshape(
            128, 8 * 128)
        wq8, dwq8 = _f8split(wq_img * WS)
        wkv8, dwkv8 = _f8split(wkv_img * WS)
        wu8h, dwu8h = _f8split(wu_img * WS)
        cosv = np.cos(th)  # [hd, pos]
        sinv = np.sin(th)
        # token-layout tables: [p, jj*128 + d] = f(theta_d * (jj*128+p))
        cosvT = cosv.T.reshape(NB, 128, 128).transpose(1, 0, 2).reshape(128, N)
        sinvT = sinv.T.reshape(NB, 128, 128).transpose(1, 0, 2).reshape(128, N)
        m0 = (np.arange(128)[:, None] <= np.arange(128)[None, :]).astype(np.float32)
        m3 = np.concatenate(
            [m0 * es1, m0 * es1, np.full((128, 128), es1, np.float32)], axis=1)
        bkv_img = np.broadcast_to(
            np.concatenate([qkvu_b[1 * D:2 * D][sl], qkvu_b[2 * D:3 * D][sl]] * 2),
            (128, 512))
        bkvid_img = np.concatenate([bkv_img, ident], axis=1)
        bqes_img = np.stack(
            [qkvu_b[0 * D:1 * D][sl], np.full(128, es1, np.float32)], axis=1)
        in_maps_a.append({
            "xpk": xpk,
            "wpk": np.ascontiguousarray(
                np.concatenate([wq8, wkv8, wu8h, dwq8, dwkv8, dwu8h], axis=1)),
            "tabT": _bf(np.concatenate([cosv, sinv], axis=1)),
            "tabTT": _bf(np.concatenate([cosvT, sinvT], axis=1)),
            "bqm": np.ascontiguousarray(
                np.concatenate([bqes_img, m3], axis=1)),
            "bkvid": _bf(bkvid_img),
        })

    if "a" not in _CACHE:
        _CACHE["a"] = build_kernel_a()
    nca = _CACHE["a"]
    if not getattr(nca, "_wsplit_done", False):
        _split_multi_waits(nca)
        nca._wsplit_done = True
    res_a = run_bass_kernel_spmd(nca, in_maps_a, list(range(NC))).results

    # reshard: core d of kernel B gets flat tokens [d*512, (d+1)*512).
    # srmsnorm and the gate fold into the reshard (elementwise glue):
    #   z = (u + bu) * y * rsqrt(mean(y^2) + eps), split into fp8 hi/lo
    NT = B * N // NC
    ys = np.stack([np.asarray(res_a[h]["y"]).astype(np.float32)
                   for h in range(H)])  # [H, B, 128, N]
    us = np.stack([np.asarray(res_a[h]["u"]).astype(np.float32)
                   for h in range(H)])  # [H, B, 128, N]
    ms = np.einsum("hbdn,hbdn->bn", ys, ys) * (1.0 / D)
    rs = 1.0 / np.sqrt(ms + EPS)  # [B, N]
    bu_f = qkvu_b[3 * D:4 * D].reshape(H, 128)  # [h, p]
    # z in [H, B, 128, N] head-parallel layout
    zf = (us + bu_f[:, None, :, None]) * ys * rs[None, :, None, :]
    wo_img = out_w.reshape(8, 128, 8, 128).transpose(3, 0, 2, 1).reshape(
        128, 64 * 128)  # [p, (oc*8+e)*128+o] = wo[oc*128+o, e*128+p]
    wo8, dwo8 = _f8split(wo_img * WS)

    in_maps_b = []
    for d in range(NC):
        bb, off = d // 4, (d % 4) * NT
        # z image [p, e*NT + t] = z[token off+t, e*128+p] = zf[e, bb, p, ...]
        z_img = np.ascontiguousarray(
            zf[:, bb, :, off:off + NT].transpose(1, 0, 2).reshape(128, 8 * NT))
        z8, dz8 = _f8split(z_img)
        in_maps_b.append({
            "z8": z8,
            "dz8": dz8,
            "wo8": wo8,
            "dwo8": dwo8,
        })

    if "b" not in _CACHE:
        _CACHE["b"] = build_kernel_b()
    ncb = _CACHE["b"]
    if not getattr(ncb, "_wsplit_done", False):
        _split_multi_waits(ncb)
        ncb._wsplit_done = True
    res_b = run_bass_kernel_spmd(ncb, in_maps_b, list(range(NC))).results

    out = np.empty((B * N, D), np.float32)
    for d in range(NC):
        o = np.asarray(res_b[d]["outT"]).astype(np.float32) * (1.0 / WS)
        out[d * NT:(d + 1) * NT] = o.reshape(128, 8, NT).transpose(
            2, 1, 0).reshape(NT, D)
    out += out_b[None, :]
    return out.reshape(B, N, D)
